# revision 20
# baseline (speedup 1.0000x reference)
"""ChebNet (K=4, two layers, log_softmax) on 8 Trainium2 NeuronCores.

Self-contained: takes FULL inputs, shards by destination node internally,
runs a single SPMD Bass kernel on cores 0-7 (ncfw AllGather between
propagation hops), returns the FULL [N, 32] output.

Math (Horner re-association so propagations happen at output width):
    y = sum_k T_k(L) x @ W[k] + b,  T_k Chebyshev,  L = -D^-1/2 A D^-1/2
      = U0 + L(U1 + L(U2 + L U3)),  U_j = x @ V_j
    V0 = W0 - W2, V1 = W1 - 3 W3, V2 = 2 W2, V3 = 4 W3
Scaled-space recurrence (tables hold S_hat = dis * S, dis = rsqrt degree):
    S_hat3 = dis*U3;  S_hat_j = dis*U_j - dis^2 * (A S_hat_{j+1})
    final: y = U_0 + b - dis * (A S_hat_1)

Per hop, A S_hat is an edge-gather + segment-sum:
    SWDGE dma_gather calls round-robined over 4 Q7 queues (the queue pairs
    run concurrently: ~4x single-queue descriptor-gen throughput) ->
    bf16 one-hot (seg == iota) built on VectorE ->
    TensorE bf16 matmul accumulation into PSUM per 128-dest tile.

Tables are bf16 with q nodes packed per 256-byte gather row (q=2 for the
64-wide layer-1 hops, q=4 for the 32-wide layer-2 hops): halves/quarters
the AllGather traffic, shrinks the int16 bucket count (2 resp. 1), and the
matmul rhs simply slices the right w-wide piece by the chunk's parity
class.  Edge chunks are scheduled per (group-of-8-dest-tiles, bucket,
parity) cell with tile-straddling one-hot pairs, so padding is ~5% instead
of ~25%.
"""

import sys

if "/opt/trn_rl_repo" not in sys.path:
    sys.path.insert(0, "/opt/trn_rl_repo")

import math
from contextlib import ExitStack
from dataclasses import dataclass, field

import numpy as np

P = 128
GCH = 8        # chunks per dma_gather call (ring limit)
OB = 32        # one-hot pairs per DVE batch
GT = 8         # dest tiles per psum group
NQ = 4         # SWDGE queues
N_NODES = 100000
N_CORES = 8
CIN, HID, OUT = 128, 64, 32


@dataclass
class LayerSched:
    q: int                 # nodes per 256B gather row
    w: int                 # data width (channels)
    nb: int                # buckets
    buckrows: int          # gather rows per bucket
    nch: int = 0           # total chunks
    npair: int = 0         # total one-hot pairs
    # per group g: list of calls (bucket, chunk0, glen)
    calls: list = field(default_factory=list)
    # per group g: list of pairs (chunk, tile_local, segcol, par, start, stop)
    pairs: list = field(default_factory=list)


@dataclass
class Cfg:
    n: int
    m: int
    b: int = 0
    t: int = 0
    bp: int = 0
    ng: int = 0
    nch: int = 0
    L: list = field(default_factory=list)  # [LayerSched x2]

    def finish(self):
        assert self.n % self.m == 0
        self.b = self.n // self.m
        self.t = (self.b + P - 1) // P
        self.bp = self.t * P
        self.ng = (self.t + GT - 1) // GT


def _layer_sched(cfg: Cfg, row, col, q, w):
    """Build the edge-stream schedule for one layer (shared across cores)."""
    m, b, bp, t, ng = cfg.m, cfg.b, cfg.bp, cfg.t, cfg.ng
    ntab = m * bp
    rows_q = ntab // q
    nb = math.ceil(rows_q / 32767)
    buckrows = rows_q // nb
    assert buckrows <= 32767 and rows_q % nb == 0

    ls = LayerSched(q=q, w=w, nb=nb, buckrows=buckrows)
    # cell = (tile, bucket, parity): chunks are tile-pure (one matmul per
    # chunk) and bucket/parity-pure (one gather base / rhs slice per chunk).
    # Tile-major cell order makes matmul consumption == gather issue order
    # AND keeps each tile's PSUM accumulation group contiguous.
    ncell = t * nb * q

    per_core = []
    counts = np.zeros((m, ncell), dtype=np.int64)
    for c in range(m):
        sel = (row >= c * b) & (row < (c + 1) * b)
        d = (row[sel] - c * b).astype(np.int64)
        s = col[sel].astype(np.int64)
        trow = (s // b) * bp + (s % b)
        pr = trow // q
        par = trow % q
        buck = pr // buckrows
        lidx = pr - buck * buckrows
        tile = d >> 7
        cid = (tile * nb + buck) * q + par
        order = np.lexsort((tile, cid))
        per_core.append((d[order], lidx[order], cid[order], tile[order]))
        counts[c] = np.bincount(cid, minlength=ncell)

    kcell = np.array(
        [math.ceil(int(counts[:, i].max()) / P) for i in range(ncell)],
        dtype=np.int64,
    )

    # global chunk layout: cells in cid order; chunk -> (g, buck, par)
    cell_chunk0 = np.concatenate([[0], np.cumsum(kcell)])
    nch = int(cell_chunk0[-1])
    ls.nch = nch

    # per-core slot streams
    S = nch * P
    idx = np.zeros((m, S), dtype=np.int32)
    dloc = np.full((m, S), -1, dtype=np.int64)   # dest local id, -1 pad
    dtile = np.full((m, S), -1, dtype=np.int64)
    for c in range(m):
        d, lidx, cid, tile = per_core[c]
        pos_in_cell = np.arange(d.size) - np.concatenate(
            [[0], np.cumsum(counts[c])]
        )[cid]
        slot = cell_chunk0[cid] * P + pos_in_cell
        idx[c, slot] = lidx
        dloc[c, slot] = d
        dtile[c, slot] = tile

    # calls (bucket-pure GCH windows over bucket runs) and one matmul
    # "pair" per chunk, in chunk order
    for g in range(ng):
        gcalls = []
        gpairs = []
        tcnt = min(GT, t - g * GT)
        for ttl in range(tcnt):
            tile = g * GT + ttl
            for buck in range(nb):
                # run of q parity cells sharing this (tile, bucket)
                c0 = int(cell_chunk0[(tile * nb + buck) * q])
                c1 = int(
                    cell_chunk0[(tile * nb + buck) * q + q - 1]
                    + kcell[(tile * nb + buck) * q + q - 1]
                )
                for w0 in range(c0, c1, GCH):
                    gcalls.append((buck, w0, min(GCH, c1 - w0)))
                for par in range(q):
                    cid = (tile * nb + buck) * q + par
                    k0 = int(cell_chunk0[cid])
                    k1 = int(cell_chunk0[cid] + kcell[cid])
                    for k in range(k0, k1):
                        gpairs.append([k, ttl, par, 0, False, False])
        first = {}
        last = {}
        for i, e in enumerate(gpairs):
            key = e[1]
            if key not in first:
                first[key] = i
            last[key] = i
        for key, i in first.items():
            gpairs[i][4] = True
        for key, i in last.items():
            gpairs[i][5] = True
        ls.calls.append(gcalls)
        ls.pairs.append(gpairs)

    ls.npair = sum(len(p) for p in ls.pairs)
    ls._idx, ls._dloc, ls._dtile = idx, dloc, dtile

    # per-core seg matrix [P, npair] and idx16 stream
    seg_all, idx_all = [], []
    for c in range(m):
        seg = np.full((P, ls.npair), -1.0, dtype=np.float32)
        colp = 0
        for g in range(ng):
            for k, ttl, par, cc, st, sp in ls.pairs[g]:
                tt = ttl + g * GT
                tl = dtile[c, k * P : (k + 1) * P]
                dl = dloc[c, k * P : (k + 1) * P]
                mask = tl == tt
                seg[mask, colp] = (dl[mask] & 127).astype(np.float32)
                colp += 1
        seg_all.append(np.ascontiguousarray(seg))

        # idx16: slot i -> partition i%16 col i//16, replicated x8
        i16 = idx[c].astype(np.int16).reshape(S // 16, 16).T
        idx_all.append(np.ascontiguousarray(np.tile(i16, (8, 1))))
    return ls, idx_all, seg_all


def preprocess(edge_index: np.ndarray, cfg: Cfg):
    row = np.asarray(edge_index[0], dtype=np.int64)
    col = np.asarray(edge_index[1], dtype=np.int64)
    deg = np.bincount(row, minlength=cfg.n).astype(np.float32)

    l1, idx1, seg1 = _layer_sched(cfg, row, col, q=2, w=HID)
    l2, idx2, seg2 = _layer_sched(cfg, row, col, q=4, w=OUT)
    cfg.L = [l1, l2]
    cfg.nch = l1.nch

    degt_all = []
    for c in range(cfg.m):
        degb = np.zeros(cfg.bp, dtype=np.float32)
        degb[: cfg.b] = deg[c * cfg.b : (c + 1) * cfg.b]
        degt_all.append(np.ascontiguousarray(degb.reshape(cfg.t, P).T))
    return (idx1, seg1, idx2, seg2), degt_all


def build_program(cfg: Cfg):
    import concourse.bass as bass
    import concourse.tile as tile
    from concourse import bacc, mybir

    f32 = mybir.dt.float32
    bf16 = mybir.dt.bfloat16
    i16 = mybir.dt.int16
    m, b, T, bp, ng = cfg.m, cfg.b, cfg.t, cfg.bp, cfg.ng
    NTAB = m * bp
    l1, l2 = cfg.L

    nc = bacc.Bacc(
        "TRN2", target_bir_lowering=False, debug=False, num_devices=m,
        num_swdge_queues=NQ,
    )

    # ---- I/O ----
    x_t = nc.dram_tensor("xT_blk", [CIN, bp], bf16, kind="ExternalInput")
    w1_t = nc.dram_tensor("W1", [4, CIN, HID], f32, kind="ExternalInput")
    b1_t = nc.dram_tensor("b1", [HID], f32, kind="ExternalInput")
    w2_t = nc.dram_tensor("W2", [4, HID, OUT], f32, kind="ExternalInput")
    b2_t = nc.dram_tensor("b2", [OUT], f32, kind="ExternalInput")
    degt_t = nc.dram_tensor("deg_t", [P, T], f32, kind="ExternalInput")
    idx1_t = nc.dram_tensor("idx1", [P, l1.nch * 8], i16, kind="ExternalInput")
    seg1_t = nc.dram_tensor("seg1", [P, l1.npair], f32, kind="ExternalInput")
    idx2_t = nc.dram_tensor("idx2", [P, l2.nch * 8], i16, kind="ExternalInput")
    seg2_t = nc.dram_tensor("seg2", [P, l2.npair], f32, kind="ExternalInput")
    y_t = nc.dram_tensor("y_blk", [bp, OUT], f32, kind="ExternalOutput")
    import os
    dbg = os.environ.get("KDBG", "0") == "1"
    if dbg:
        dbg_blk = nc.dram_tensor("dbg_blk", [bp // l1.q, 128], bf16, kind="ExternalOutput")
        dbg_tab = nc.dram_tensor("dbg_tab", [NTAB // l1.q, 128], bf16, kind="ExternalOutput")
        dbg_u1 = nc.dram_tensor("dbg_u1", [3, bp, HID], f32, kind="ExternalOutput")
        dbg_h = nc.dram_tensor("dbg_h", [bp, HID], f32, kind="ExternalOutput")
        dbg_blk2 = nc.dram_tensor("dbg_blk2", [bp // l1.q, 128], bf16, kind="ExternalOutput")

    # ---- internal DRAM ----
    u1 = nc.dram_tensor("U1", [3, bp, HID], f32)
    u2 = nc.dram_tensor("U2", [3, bp, OUT], f32)
    h_t = nc.dram_tensor("h", [bp, HID], bf16)
    blks, tabs = {}, {}
    for l, ls in ((1, l1), (2, l2)):
        for j in (3, 2, 1):
            blks[(l, j)] = nc.dram_tensor(f"blk_{l}_{j}", [bp // ls.q, 128], bf16)
            tabs[(l, j)] = nc.dram_tensor(f"tab_{l}_{j}", [NTAB // ls.q, 128], bf16)

    iota_np = np.broadcast_to(
        np.tile(np.arange(P, dtype=np.float32), OB), (P, OB * P)
    ).copy()
    iota_d = nc.inline_tensor(iota_np, name="iota_rep")
    ident_d = nc.inline_tensor(np.eye(P, dtype=np.float32), name="ident")

    with ExitStack() as ctx:
        tc = ctx.enter_context(tile.TileContext(nc, num_cores=m))
        const = ctx.enter_context(tc.tile_pool(name="const", bufs=1))
        xp = ctx.enter_context(tc.tile_pool(name="xp", bufs=3))
        wp = ctx.enter_context(tc.tile_pool(name="wp", bufs=3))
        up = ctx.enter_context(tc.tile_pool(name="up", bufs=3))
        gp = ctx.enter_context(tc.tile_pool(name="gp", bufs=12))
        op = ctx.enter_context(tc.tile_pool(name="op", bufs=3))
        ep = ctx.enter_context(tc.tile_pool(name="ep", bufs=2))
        pst = ctx.enter_context(tc.tile_pool(name="pst", bufs=2, space="PSUM"))
        psu = ctx.enter_context(tc.tile_pool(name="psu", bufs=2, space="PSUM"))
        psa = ctx.enter_context(tc.tile_pool(name="psa", bufs=2, space="PSUM"))

        # ---- constants ----
        iota_s = const.tile([P, OB * P], f32)
        nc.sync.dma_start(iota_s[:], iota_d[:, :])
        ident_s = const.tile([P, P], f32)
        nc.sync.dma_start(ident_s[:], ident_d[:, :])

        idx1_s = const.tile([P, l1.nch * 8], i16)
        nc.sync.dma_start(idx1_s[:], idx1_t[:, :])
        seg1_s = const.tile([P, l1.npair], f32)
        nc.sync.dma_start(seg1_s[:], seg1_t[:, :])
        idx2_s = const.tile([P, l2.nch * 8], i16)
        nc.sync.dma_start(idx2_s[:], idx2_t[:, :])
        seg2_s = const.tile([P, l2.npair], f32)
        nc.sync.dma_start(seg2_s[:], seg2_t[:, :])

        # V1cat [CIN, 4, HID], V2cat [HID, 4, OUT]
        def vcat(w_t, cl, w):
            ws = const.tile([cl, 4, w], f32)
            nc.sync.dma_start(ws[:], w_t[:, :, :].rearrange("k p c -> p k c"))
            v = const.tile([cl, 4, w], f32)
            nc.vector.tensor_sub(v[:, 0, :], ws[:, 0, :], ws[:, 2, :])
            nc.vector.tensor_scalar(
                out=v[:, 1, :], in0=ws[:, 3, :], scalar1=-3.0, scalar2=None,
                op0=mybir.AluOpType.mult,
            )
            nc.vector.tensor_add(v[:, 1, :], v[:, 1, :], ws[:, 1, :])
            nc.vector.tensor_scalar(
                out=v[:, 2, :], in0=ws[:, 2, :], scalar1=2.0, scalar2=None,
                op0=mybir.AluOpType.mult,
            )
            nc.vector.tensor_scalar(
                out=v[:, 3, :], in0=ws[:, 3, :], scalar1=4.0, scalar2=None,
                op0=mybir.AluOpType.mult,
            )
            return v

        v1f = vcat(w1_t, CIN, HID)
        v2f = vcat(w2_t, HID, OUT)
        v1 = const.tile([CIN, 4, HID], bf16)
        nc.vector.tensor_copy(v1[:], v1f[:])
        v2 = const.tile([HID, 4, OUT], bf16)
        nc.vector.tensor_copy(v2[:], v2f[:])
        ident_b = const.tile([P, P], bf16)
        nc.vector.tensor_copy(ident_b[:], ident_s[:])

        b1s = const.tile([P, HID], f32)
        nc.sync.dma_start(b1s[:1, :], b1_t[:].rearrange("(o c) -> o c", o=1))
        nc.gpsimd.partition_broadcast(b1s[:, :], b1s[:1, :])
        b2s = const.tile([P, OUT], f32)
        nc.sync.dma_start(b2s[:1, :], b2_t[:].rearrange("(o c) -> o c", o=1))
        nc.gpsimd.partition_broadcast(b2s[:, :], b2s[:1, :])

        # ---- dis, -dis, -dis^2 in [P, T]: (p, t) = dest 128t+p ----
        degs = const.tile([P, T], f32)
        nc.sync.dma_start(degs[:], degt_t[:, :])
        dis = const.tile([P, T], f32)
        ndis = const.tile([P, T], f32)
        ndis2 = const.tile([P, T], f32)
        tmp = const.tile([P, T], f32)
        nc.vector.tensor_scalar(
            out=tmp[:], in0=degs[:], scalar1=1.0, scalar2=None,
            op0=mybir.AluOpType.max,
        )
        nc.scalar.activation(tmp[:], tmp[:], mybir.ActivationFunctionType.Sqrt)
        nc.vector.reciprocal(dis[:], tmp[:])
        nc.vector.tensor_scalar(
            out=tmp[:], in0=degs[:], scalar1=0.0, scalar2=None,
            op0=mybir.AluOpType.is_gt,
        )
        nc.vector.tensor_mul(dis[:], dis[:], tmp[:])
        nc.vector.tensor_scalar(
            out=ndis[:], in0=dis[:], scalar1=-1.0, scalar2=None,
            op0=mybir.AluOpType.mult,
        )
        nc.vector.tensor_mul(ndis2[:], dis[:], ndis[:])

        # blk row views: [bp, w] over [bp/q, 128]
        def rows_view(blk, w):
            return blk[:, :].rearrange("r (t c) -> (r t) c", c=w)

        # ---- prologue: U_j = src @ V_j; U1/U2 (j=1,2 dis-scaled, j=0 +bias)
        #      to HBM; j=3 dis-scaled -> blk (bf16) ----
        def prologue(src_hbm, cl, w, vc, u_hbm, blk_rows, bb, transpose_src):
            for k in range(T):
                if transpose_src:
                    xc = xp.tile([P, cl], bf16, tag="xc")
                    nc.sync.dma_start(xc[:], src_hbm[k * P : (k + 1) * P, :])
                    tp = pst.tile([cl, P], bf16, space="PSUM", tag="tp")
                    nc.tensor.transpose(
                        out=tp[:, :], in_=xc[:, :], identity=ident_b[:]
                    )
                    xT = wp.tile([cl, P], bf16, tag="xT")
                    nc.vector.tensor_copy(xT[:], tp[:, :])
                else:
                    xT = xp.tile([cl, P], bf16, tag="xTd")
                    nc.sync.dma_start(xT[:], src_hbm[:, k * P : (k + 1) * P])
                upsum = psu.tile([P, 4, w], f32, space="PSUM", tag="upsum")
                nc.tensor.matmul(
                    out=upsum[:].rearrange("p a c -> p (a c)"),
                    lhsT=xT[:, :],
                    rhs=vc[:].rearrange("p a c -> p (a c)"),
                    start=True, stop=True,
                )
                ut = wp.tile([P, 3, w], f32, tag="ut")
                nc.vector.tensor_add(ut[:, 0, :], upsum[:, 0, :], bb[:, :w])
                for j in (1, 2):
                    nc.vector.tensor_scalar(
                        out=ut[:, j, :], in0=upsum[:, j, :],
                        scalar1=dis[:, k : k + 1], scalar2=None,
                        op0=mybir.AluOpType.mult,
                    )
                nc.sync.dma_start(
                    u_hbm[:, k * P : (k + 1) * P, :].rearrange("a p c -> p a c"),
                    ut[:],
                )
                bt = wp.tile([P, w], bf16, tag="bt")
                nc.vector.tensor_scalar(
                    out=bt[:, :], in0=upsum[:, 3, :],
                    scalar1=dis[:, k : k + 1], scalar2=None,
                    op0=mybir.AluOpType.mult,
                )
                nc.sync.dma_start(
                    blk_rows[k * P : (k + 1) * P, :].rearrange(
                        "(a p) c -> p (a c)", p=P
                    ),
                    bt[:],
                )

        def allgather(blk, tab):
            nc.gpsimd.collective_compute(
                "AllGather",
                mybir.AluOpType.bypass,
                replica_groups=[list(range(m))],
                ins=[blk.ap().opt()],
                outs=[tab.ap().opt()],
            )

        # ---- one hop ----
        def hop(ls, idx_s, seg_s, tab, u_hbm, blk_rows, final, l):
            w = ls.w
            segbase = 0
            qrr = [0]
            for g in range(ng):
                tcnt = min(GT, T - g * GT)
                gath = {}
                for buck, w0, glen in ls.calls[g]:
                    gt = gp.tile([P, GCH, 128], bf16, tag="gath")
                    nc.gpsimd.dma_gather(
                        out_ap=gt[:, :glen, :],
                        in_ap=tab[buck * ls.buckrows : (buck + 1) * ls.buckrows, :],
                        idxs_ap=idx_s[:, w0 * 8 : (w0 + glen) * 8],
                        num_idxs=glen * P,
                        num_idxs_reg=glen * P,
                        elem_size=128,
                        queue_num=qrr[0] % NQ,
                    )
                    qrr[0] += 1
                    for j in range(glen):
                        gath[w0 + j] = (gt, j)

                psum = psa.tile([P, GT, w], f32, space="PSUM", tag="apsum")
                started = {e[1] for e in ls.pairs[g] if e[4]}
                for ttl in range(tcnt):
                    if ttl not in started:
                        nc.vector.memset(psum[:, ttl, :], 0.0)
                oneh = None
                npair_g = len(ls.pairs[g])
                for i, (k, ttl, par, cc, st, sp) in enumerate(ls.pairs[g]):
                    opos = i % OB
                    if opos == 0:
                        olen = min(OB, npair_g - i)
                        oneh = op.tile([P, OB, P], bf16, tag="oneh")
                        nc.vector.tensor_tensor(
                            out=oneh[:, :olen, :],
                            in0=iota_s[:].rearrange("p (a q) -> p a q", q=P)[
                                :, :olen, :
                            ],
                            in1=seg_s[:, segbase + i : segbase + i + olen]
                            .to_broadcast([P, olen, P]),
                            op=mybir.AluOpType.is_equal,
                        )
                    gt, slot = gath[k]
                    nc.tensor.matmul(
                        out=psum[:, ttl, :],
                        lhsT=oneh[:, opos, :],
                        rhs=gt[:, slot, par * w : (par + 1) * w],
                        start=st, stop=sp,
                    )
                segbase += npair_g

                # ---- writeout ----
                sl = slice(g * GT, g * GT + tcnt)
                rows = slice(g * GT * P, (g * GT + tcnt) * P)
                uti = up.tile([P, GT, w], f32, tag="uti")
                nc.sync.dma_start(
                    uti[:, :tcnt, :],
                    u_hbm[0 if final else ls.j, rows, :].rearrange(
                        "(a p) c -> p a c", p=P
                    ),
                )
                wt = wp.tile([P, GT, w], f32, tag="wt")
                nc.vector.tensor_tensor(
                    out=wt[:, :tcnt, :],
                    in0=psum[:, :tcnt, :],
                    in1=(ndis if final else ndis2)[:, sl].to_broadcast(
                        [P, tcnt, w]
                    ),
                    op=mybir.AluOpType.mult,
                )
                nc.vector.tensor_add(
                    wt[:, :tcnt, :], wt[:, :tcnt, :], uti[:, :tcnt, :]
                )
                if not final:
                    bt = ep.tile([P, GT, w], bf16, tag="bt")
                    nc.vector.tensor_copy(bt[:, :tcnt, :], wt[:, :tcnt, :])
                    nc.sync.dma_start(
                        blk_rows[rows, :].rearrange("(a p) c -> p a c", p=P),
                        bt[:, :tcnt, :],
                    )
                elif l == 1:
                    nc.vector.tensor_scalar(
                        out=wt[:, :tcnt, :], in0=wt[:, :tcnt, :],
                        scalar1=0.0, scalar2=None, op0=mybir.AluOpType.max,
                    )
                    bt = ep.tile([P, GT, w], bf16, tag="btr")
                    nc.vector.tensor_copy(bt[:, :tcnt, :], wt[:, :tcnt, :])
                    nc.sync.dma_start(
                        h_t[rows, :].rearrange("(a p) c -> p a c", p=P),
                        bt[:, :tcnt, :],
                    )
                else:
                    nc.vector.tensor_copy(lsm[:, sl, :], wt[:, :tcnt, :])

        lsm = const.tile([P, T, OUT], f32)

        # ================= layer 1 =================
        prologue(x_t, CIN, HID, v1, u1, rows_view(blks[(1, 3)], HID), b1s,
                 transpose_src=False)
        for j in (2, 1, 0):
            allgather(blks[(1, j + 1)], tabs[(1, j + 1)])
            l1.j = j
            hop(l1, idx1_s, seg1_s, tabs[(1, j + 1)], u1,
                rows_view(blks[(1, j)], HID) if j else None,
                final=(j == 0), l=1)

        if dbg:
            nc.sync.dma_start(dbg_blk[:, :], blks[(1, 3)][:, :])
            nc.sync.dma_start(dbg_tab[:, :], tabs[(1, 3)][:, :])
            nc.sync.dma_start(dbg_u1[:, :, :], u1[:, :, :])
            nc.sync.dma_start(dbg_h[:, :], h_t[:, :])
            nc.sync.dma_start(dbg_blk2[:, :], blks[(1, 2)][:, :])

        # ================= layer 2 =================
        prologue(h_t, HID, OUT, v2, u2, rows_view(blks[(2, 3)], OUT), b2s,
                 transpose_src=True)
        for j in (2, 1, 0):
            allgather(blks[(2, j + 1)], tabs[(2, j + 1)])
            l2.j = j
            hop(l2, idx2_s, seg2_s, tabs[(2, j + 1)], u2,
                rows_view(blks[(2, j)], OUT) if j else None,
                final=(j == 0), l=2)

        # ---- batched log_softmax epilogue over lsm [P, T, OUT] ----
        red = const.tile([P, T], f32)
        nc.vector.tensor_reduce(
            out=red[:], in_=lsm[:, :, :], axis=mybir.AxisListType.X,
            op=mybir.AluOpType.max,
        )
        nc.vector.tensor_tensor(
            out=lsm[:, :, :], in0=lsm[:, :, :],
            in1=red[:].to_broadcast([P, T, OUT]),
            op=mybir.AluOpType.subtract,
        )
        ex = const.tile([P, T, OUT], bf16)
        nc.scalar.activation(ex[:], lsm[:, :, :], mybir.ActivationFunctionType.Exp)
        nc.vector.tensor_reduce(
            out=red[:], in_=ex[:, :, :], axis=mybir.AxisListType.X,
            op=mybir.AluOpType.add,
        )
        nc.scalar.activation(red[:], red[:], mybir.ActivationFunctionType.Ln)
        nc.vector.tensor_tensor(
            out=lsm[:, :, :], in0=lsm[:, :, :],
            in1=red[:].to_broadcast([P, T, OUT]),
            op=mybir.AluOpType.subtract,
        )
        nc.sync.dma_start(
            y_t[:, :].rearrange("(a p) c -> p a c", p=P), lsm[:, :, :]
        )

    nc.compile()
    return nc


def make_in_maps(cfg: Cfg, inputs: dict, idxseg, degt_all):
    idx1, seg1, idx2, seg2 = idxseg
    import ml_dtypes

    x = np.asarray(inputs["x"], dtype=np.float32)
    maps = []
    for c in range(cfg.m):
        xb = np.zeros((cfg.bp, CIN), dtype=np.float32)
        xb[: cfg.b] = x[c * cfg.b : (c + 1) * cfg.b]
        xT = np.ascontiguousarray(xb.T).astype(ml_dtypes.bfloat16)
        maps.append(
            {
                "xT_blk": xT,
                "W1": np.asarray(inputs["W1"], dtype=np.float32),
                "b1": np.asarray(inputs["b1"], dtype=np.float32),
                "W2": np.asarray(inputs["W2"], dtype=np.float32),
                "b2": np.asarray(inputs["b2"], dtype=np.float32),
                "deg_t": degt_all[c],
                "idx1": idx1[c],
                "seg1": seg1[c],
                "idx2": idx2[c],
                "seg2": seg2[c],
            }
        )
    return maps


def kernel(**inputs) -> np.ndarray:
    from concourse import bass_utils

    cfg = Cfg(n=N_NODES, m=N_CORES)
    cfg.finish()
    edge_index = np.asarray(inputs["edge_index"])
    idxseg, degt_all = preprocess(edge_index, cfg)
    nc = build_program(cfg)
    in_maps = make_in_maps(cfg, inputs, idxseg, degt_all)
    res = bass_utils.run_bass_kernel_spmd(nc, in_maps, core_ids=list(range(cfg.m)))
    out = np.concatenate(
        [res.results[c]["y_blk"][: cfg.b] for c in range(cfg.m)], axis=0
    )
    return out.astype(np.float32)


# revision 21
# speedup vs baseline: 2.0025x; 2.0025x over previous
"""ChebNet (K=4, two layers, log_softmax) on 8 Trainium2 NeuronCores.

Self-contained: takes FULL inputs, shards by destination node internally,
runs a single SPMD Bass kernel on cores 0-7 (ncfw AllGather between
propagation hops), returns the FULL [N, 32] output.

Math (Horner re-association so propagations happen at output width):
    y = sum_k T_k(L) x @ W[k] + b,  T_k Chebyshev,  L = -D^-1/2 A D^-1/2
      = U0 + L(U1 + L(U2 + L U3)),  U_j = x @ V_j
    V0 = W0 - W2, V1 = W1 - 3 W3, V2 = 2 W2, V3 = 4 W3
Scaled-space recurrence (tables hold S_hat = dis * S, dis = rsqrt degree):
    S_hat3 = dis*U3;  S_hat_j = dis*U_j - dis^2 * (A S_hat_{j+1})
    final: y = U_0 + b - dis * (A S_hat_1)

Per hop, A S_hat is an edge-gather + segment-sum:
    SWDGE dma_gather calls round-robined over 4 Q7 queues (the queue pairs
    run concurrently: ~4x single-queue descriptor-gen throughput) ->
    bf16 one-hot (seg == iota) built on VectorE ->
    TensorE bf16 matmul accumulation into PSUM per 128-dest tile.

Tables are bf16 with q nodes packed per 256-byte gather row (q=2 for the
64-wide layer-1 hops, q=4 for the 32-wide layer-2 hops): halves/quarters
the AllGather traffic, shrinks the int16 bucket count (2 resp. 1), and the
matmul rhs simply slices the right w-wide piece by the chunk's parity
class.  Edge chunks are scheduled per (group-of-8-dest-tiles, bucket,
parity) cell with tile-straddling one-hot pairs, so padding is ~5% instead
of ~25%.
"""

import sys

if "/opt/trn_rl_repo" not in sys.path:
    sys.path.insert(0, "/opt/trn_rl_repo")

import math
from contextlib import ExitStack
from dataclasses import dataclass, field

import numpy as np

P = 128
GCH = 8        # chunks per dma_gather call (ring limit)
OB = 32        # one-hot pairs per DVE batch
GT = 8         # dest tiles per psum group
NQ = 4         # SWDGE queues
N_NODES = 100000
N_CORES = 8
CIN, HID, OUT = 128, 64, 32


@dataclass
class LayerSched:
    q: int                 # nodes per 256B gather row
    w: int                 # data width (channels)
    nb: int                # buckets
    buckrows: int          # gather rows per bucket
    nch: int = 0           # total chunks
    npair: int = 0         # total one-hot pairs
    # per group g: list of calls (bucket, chunk0, glen)
    calls: list = field(default_factory=list)
    # per group g: list of pairs (chunk, tile_local, segcol, par, start, stop)
    pairs: list = field(default_factory=list)


@dataclass
class Cfg:
    n: int
    m: int
    b: int = 0
    t: int = 0
    bp: int = 0
    ng: int = 0
    nch: int = 0
    L: list = field(default_factory=list)  # [LayerSched x2]

    def finish(self):
        assert self.n % self.m == 0
        self.b = self.n // self.m
        self.t = (self.b + P - 1) // P
        self.bp = self.t * P
        self.ng = (self.t + GT - 1) // GT


def _layer_sched(cfg: Cfg, row, col, q, w):
    """Build the edge-stream schedule for one layer (shared across cores)."""
    m, b, bp, t, ng = cfg.m, cfg.b, cfg.bp, cfg.t, cfg.ng
    ntab = m * bp
    rows_q = ntab // q
    nb = math.ceil(rows_q / 32767)
    buckrows = rows_q // nb
    assert buckrows <= 32767 and rows_q % nb == 0

    ls = LayerSched(q=q, w=w, nb=nb, buckrows=buckrows)
    # cell = (group, bucket, parity): fewer chunks (padding amortized over
    # 8 tiles) and fewer, fatter gather calls (per-call overhead is the
    # dominant SWDGE cost at 4-queue parallelism)
    ncell = ng * nb * q

    per_core = []
    counts = np.zeros((m, ncell), dtype=np.int64)
    for c in range(m):
        sel = (row >= c * b) & (row < (c + 1) * b)
        d = (row[sel] - c * b).astype(np.int64)
        s = col[sel].astype(np.int64)
        trow = (s // b) * bp + (s % b)
        pr = trow // q
        par = trow % q
        buck = pr // buckrows
        lidx = pr - buck * buckrows
        tile = d >> 7
        g = tile // GT
        cid = (g * nb + buck) * q + par
        order = np.lexsort((tile, cid))
        per_core.append((d[order], lidx[order], cid[order], tile[order]))
        counts[c] = np.bincount(cid, minlength=ncell)

    kcell = np.array(
        [math.ceil(int(counts[:, i].max()) / P) for i in range(ncell)],
        dtype=np.int64,
    )

    # global chunk layout: cells in cid order; chunk -> (g, buck, par)
    cell_chunk0 = np.concatenate([[0], np.cumsum(kcell)])
    nch = int(cell_chunk0[-1])
    ls.nch = nch

    # per-core slot streams
    S = nch * P
    idx = np.zeros((m, S), dtype=np.int32)
    dloc = np.full((m, S), -1, dtype=np.int64)   # dest local id, -1 pad
    dtile = np.full((m, S), -1, dtype=np.int64)
    for c in range(m):
        d, lidx, cid, tile = per_core[c]
        pos_in_cell = np.arange(d.size) - np.concatenate(
            [[0], np.cumsum(counts[c])]
        )[cid]
        slot = cell_chunk0[cid] * P + pos_in_cell
        idx[c, slot] = lidx
        dloc[c, slot] = d
        dtile[c, slot] = tile

    # calls: bucket-pure GCH windows per (group, bucket); pairs: per chunk,
    # union tile range over cores, TILE-MAJOR (contiguous psum accumulation
    # groups per tile)
    for g in range(ng):
        gcalls = []
        gpairs = []
        tcnt = min(GT, t - g * GT)
        for buck in range(nb):
            c0 = int(cell_chunk0[(g * nb + buck) * q])
            cl_ = (g * nb + buck) * q + q - 1
            c1 = int(cell_chunk0[cl_] + kcell[cl_])
            for w0 in range(c0, c1, GCH):
                gcalls.append((buck, w0, min(GCH, c1 - w0)))
        for par in range(q):
            for buck in range(nb):
                cid = (g * nb + buck) * q + par
                k0 = int(cell_chunk0[cid])
                k1 = int(cell_chunk0[cid] + kcell[cid])
                for k in range(k0, k1):
                    tl = dtile[:, k * P : (k + 1) * P]
                    real = tl >= 0
                    if not real.any():
                        continue
                    lo = int(tl[real].min())
                    hi = int(tl[real].max())
                    for tt in range(lo, hi + 1):
                        gpairs.append([k, tt - g * GT, par, 0, False, False])
        gpairs.sort(key=lambda e: (e[1], e[0]))
        first = {}
        last = {}
        for i, e in enumerate(gpairs):
            key = e[1]
            if key not in first:
                first[key] = i
            last[key] = i
        for key, i in first.items():
            gpairs[i][4] = True
        for key, i in last.items():
            gpairs[i][5] = True
        ls.calls.append(gcalls)
        ls.pairs.append(gpairs)

    ls.npair = sum(len(p) for p in ls.pairs)
    ls._idx, ls._dloc, ls._dtile = idx, dloc, dtile

    # per-core seg matrix [P, npair] and idx16 stream
    seg_all, idx_all = [], []
    for c in range(m):
        seg = np.full((P, ls.npair), -1.0, dtype=np.float32)
        colp = 0
        for g in range(ng):
            for k, ttl, par, cc, st, sp in ls.pairs[g]:
                tt = ttl + g * GT
                tl = dtile[c, k * P : (k + 1) * P]
                dl = dloc[c, k * P : (k + 1) * P]
                mask = tl == tt
                seg[mask, colp] = (dl[mask] & 127).astype(np.float32)
                colp += 1
        seg_all.append(np.ascontiguousarray(seg))

        # idx16: slot i -> partition i%16 col i//16, replicated x8
        i16 = idx[c].astype(np.int16).reshape(S // 16, 16).T
        idx_all.append(np.ascontiguousarray(np.tile(i16, (8, 1))))
    return ls, idx_all, seg_all


def preprocess(edge_index: np.ndarray, cfg: Cfg):
    row = np.asarray(edge_index[0], dtype=np.int64)
    col = np.asarray(edge_index[1], dtype=np.int64)
    deg = np.bincount(row, minlength=cfg.n).astype(np.float32)

    l1, idx1, seg1 = _layer_sched(cfg, row, col, q=2, w=HID)
    l2, idx2, seg2 = _layer_sched(cfg, row, col, q=4, w=OUT)
    cfg.L = [l1, l2]
    cfg.nch = l1.nch

    degt_all = []
    for c in range(cfg.m):
        degb = np.zeros(cfg.bp, dtype=np.float32)
        degb[: cfg.b] = deg[c * cfg.b : (c + 1) * cfg.b]
        degt_all.append(np.ascontiguousarray(degb.reshape(cfg.t, P).T))
    return (idx1, seg1, idx2, seg2), degt_all


def build_program(cfg: Cfg):
    import concourse.bass as bass
    import concourse.tile as tile
    from concourse import bacc, mybir

    f32 = mybir.dt.float32
    bf16 = mybir.dt.bfloat16
    i16 = mybir.dt.int16
    m, b, T, bp, ng = cfg.m, cfg.b, cfg.t, cfg.bp, cfg.ng
    NTAB = m * bp
    l1, l2 = cfg.L

    nc = bacc.Bacc(
        "TRN2", target_bir_lowering=False, debug=False, num_devices=m,
        num_swdge_queues=NQ,
    )

    # ---- I/O ----
    x_t = nc.dram_tensor("xT_blk", [CIN, bp], bf16, kind="ExternalInput")
    w1_t = nc.dram_tensor("W1", [4, CIN, HID], f32, kind="ExternalInput")
    b1_t = nc.dram_tensor("b1", [HID], f32, kind="ExternalInput")
    w2_t = nc.dram_tensor("W2", [4, HID, OUT], f32, kind="ExternalInput")
    b2_t = nc.dram_tensor("b2", [OUT], f32, kind="ExternalInput")
    degt_t = nc.dram_tensor("deg_t", [P, T], f32, kind="ExternalInput")
    idx1_t = nc.dram_tensor("idx1", [P, l1.nch * 8], i16, kind="ExternalInput")
    seg1_t = nc.dram_tensor("seg1", [P, l1.npair], f32, kind="ExternalInput")
    idx2_t = nc.dram_tensor("idx2", [P, l2.nch * 8], i16, kind="ExternalInput")
    seg2_t = nc.dram_tensor("seg2", [P, l2.npair], f32, kind="ExternalInput")
    y_t = nc.dram_tensor("y_blk", [bp, OUT], f32, kind="ExternalOutput")
    import os
    dbg = os.environ.get("KDBG", "0") == "1"
    if dbg:
        dbg_blk = nc.dram_tensor("dbg_blk", [bp // l1.q, 128], bf16, kind="ExternalOutput")
        dbg_tab = nc.dram_tensor("dbg_tab", [NTAB // l1.q, 128], bf16, kind="ExternalOutput")
        dbg_u1 = nc.dram_tensor("dbg_u1", [3, bp, HID], f32, kind="ExternalOutput")
        dbg_h = nc.dram_tensor("dbg_h", [bp, HID], f32, kind="ExternalOutput")
        dbg_blk2 = nc.dram_tensor("dbg_blk2", [bp // l1.q, 128], bf16, kind="ExternalOutput")

    # ---- internal DRAM ----
    u1 = nc.dram_tensor("U1", [3, bp, HID], f32)
    u2 = nc.dram_tensor("U2", [3, bp, OUT], f32)
    h_t = nc.dram_tensor("h", [bp, HID], bf16)
    blks, tabs = {}, {}
    for l, ls in ((1, l1), (2, l2)):
        for j in (3, 2, 1):
            blks[(l, j)] = nc.dram_tensor(f"blk_{l}_{j}", [bp // ls.q, 128], bf16)
            tabs[(l, j)] = nc.dram_tensor(f"tab_{l}_{j}", [NTAB // ls.q, 128], bf16)

    iota_np = np.broadcast_to(
        np.tile(np.arange(P, dtype=np.float32), OB), (P, OB * P)
    ).copy()
    iota_d = nc.inline_tensor(iota_np, name="iota_rep")
    ident_d = nc.inline_tensor(np.eye(P, dtype=np.float32), name="ident")

    with ExitStack() as ctx:
        tc = ctx.enter_context(tile.TileContext(nc, num_cores=m))
        const = ctx.enter_context(tc.tile_pool(name="const", bufs=1))
        xp = ctx.enter_context(tc.tile_pool(name="xp", bufs=3))
        wp = ctx.enter_context(tc.tile_pool(name="wp", bufs=3))
        up = ctx.enter_context(tc.tile_pool(name="up", bufs=3))
        gp = ctx.enter_context(tc.tile_pool(name="gp", bufs=24))
        op = ctx.enter_context(tc.tile_pool(name="op", bufs=3))
        ep = ctx.enter_context(tc.tile_pool(name="ep", bufs=2))
        pst = ctx.enter_context(tc.tile_pool(name="pst", bufs=2, space="PSUM"))
        psu = ctx.enter_context(tc.tile_pool(name="psu", bufs=2, space="PSUM"))
        psa = ctx.enter_context(tc.tile_pool(name="psa", bufs=2, space="PSUM"))

        # ---- constants ----
        iota_s = const.tile([P, OB * P], f32)
        nc.sync.dma_start(iota_s[:], iota_d[:, :])
        ident_s = const.tile([P, P], f32)
        nc.sync.dma_start(ident_s[:], ident_d[:, :])

        idx1_s = const.tile([P, l1.nch * 8], i16)
        nc.sync.dma_start(idx1_s[:], idx1_t[:, :])
        seg1_s = const.tile([P, l1.npair], f32)
        nc.sync.dma_start(seg1_s[:], seg1_t[:, :])
        idx2_s = const.tile([P, l2.nch * 8], i16)
        nc.sync.dma_start(idx2_s[:], idx2_t[:, :])
        seg2_s = const.tile([P, l2.npair], f32)
        nc.sync.dma_start(seg2_s[:], seg2_t[:, :])

        # V1cat [CIN, 4, HID], V2cat [HID, 4, OUT]
        def vcat(w_t, cl, w):
            ws = const.tile([cl, 4, w], f32)
            nc.sync.dma_start(ws[:], w_t[:, :, :].rearrange("k p c -> p k c"))
            v = const.tile([cl, 4, w], f32)
            nc.vector.tensor_sub(v[:, 0, :], ws[:, 0, :], ws[:, 2, :])
            nc.vector.tensor_scalar(
                out=v[:, 1, :], in0=ws[:, 3, :], scalar1=-3.0, scalar2=None,
                op0=mybir.AluOpType.mult,
            )
            nc.vector.tensor_add(v[:, 1, :], v[:, 1, :], ws[:, 1, :])
            nc.vector.tensor_scalar(
                out=v[:, 2, :], in0=ws[:, 2, :], scalar1=2.0, scalar2=None,
                op0=mybir.AluOpType.mult,
            )
            nc.vector.tensor_scalar(
                out=v[:, 3, :], in0=ws[:, 3, :], scalar1=4.0, scalar2=None,
                op0=mybir.AluOpType.mult,
            )
            return v

        v1f = vcat(w1_t, CIN, HID)
        v2f = vcat(w2_t, HID, OUT)
        v1 = const.tile([CIN, 4, HID], bf16)
        nc.vector.tensor_copy(v1[:], v1f[:])
        v2 = const.tile([HID, 4, OUT], bf16)
        nc.vector.tensor_copy(v2[:], v2f[:])
        ident_b = const.tile([P, P], bf16)
        nc.vector.tensor_copy(ident_b[:], ident_s[:])

        b1s = const.tile([P, HID], f32)
        nc.sync.dma_start(b1s[:1, :], b1_t[:].rearrange("(o c) -> o c", o=1))
        nc.gpsimd.partition_broadcast(b1s[:, :], b1s[:1, :])
        b2s = const.tile([P, OUT], f32)
        nc.sync.dma_start(b2s[:1, :], b2_t[:].rearrange("(o c) -> o c", o=1))
        nc.gpsimd.partition_broadcast(b2s[:, :], b2s[:1, :])

        # ---- dis, -dis, -dis^2 in [P, T]: (p, t) = dest 128t+p ----
        degs = const.tile([P, T], f32)
        nc.sync.dma_start(degs[:], degt_t[:, :])
        dis = const.tile([P, T], f32)
        ndis = const.tile([P, T], f32)
        ndis2 = const.tile([P, T], f32)
        tmp = const.tile([P, T], f32)
        nc.vector.tensor_scalar(
            out=tmp[:], in0=degs[:], scalar1=1.0, scalar2=None,
            op0=mybir.AluOpType.max,
        )
        nc.scalar.activation(tmp[:], tmp[:], mybir.ActivationFunctionType.Sqrt)
        nc.vector.reciprocal(dis[:], tmp[:])
        nc.vector.tensor_scalar(
            out=tmp[:], in0=degs[:], scalar1=0.0, scalar2=None,
            op0=mybir.AluOpType.is_gt,
        )
        nc.vector.tensor_mul(dis[:], dis[:], tmp[:])
        nc.vector.tensor_scalar(
            out=ndis[:], in0=dis[:], scalar1=-1.0, scalar2=None,
            op0=mybir.AluOpType.mult,
        )
        nc.vector.tensor_mul(ndis2[:], dis[:], ndis[:])

        # blk row views: [bp, w] over [bp/q, 128]
        def rows_view(blk, w):
            return blk[:, :].rearrange("r (t c) -> (r t) c", c=w)

        # ---- prologue: U_j = src @ V_j; U1/U2 (j=1,2 dis-scaled, j=0 +bias)
        #      to HBM; j=3 dis-scaled -> blk (bf16) ----
        def prologue(src_hbm, cl, w, vc, u_hbm, blk_rows, bb, transpose_src):
            for k in range(T):
                if transpose_src:
                    xc = xp.tile([P, cl], bf16, tag="xc")
                    nc.sync.dma_start(xc[:], src_hbm[k * P : (k + 1) * P, :])
                    tp = pst.tile([cl, P], bf16, space="PSUM", tag="tp")
                    nc.tensor.transpose(
                        out=tp[:, :], in_=xc[:, :], identity=ident_b[:]
                    )
                    xT = wp.tile([cl, P], bf16, tag="xT")
                    nc.vector.tensor_copy(xT[:], tp[:, :])
                else:
                    xT = xp.tile([cl, P], bf16, tag="xTd")
                    nc.sync.dma_start(xT[:], src_hbm[:, k * P : (k + 1) * P])
                upsum = psu.tile([P, 4, w], f32, space="PSUM", tag="upsum")
                nc.tensor.matmul(
                    out=upsum[:].rearrange("p a c -> p (a c)"),
                    lhsT=xT[:, :],
                    rhs=vc[:].rearrange("p a c -> p (a c)"),
                    start=True, stop=True,
                )
                ut = wp.tile([P, 3, w], f32, tag="ut")
                nc.vector.tensor_add(ut[:, 0, :], upsum[:, 0, :], bb[:, :w])
                for j in (1, 2):
                    nc.vector.tensor_scalar(
                        out=ut[:, j, :], in0=upsum[:, j, :],
                        scalar1=dis[:, k : k + 1], scalar2=None,
                        op0=mybir.AluOpType.mult,
                    )
                nc.sync.dma_start(
                    u_hbm[:, k * P : (k + 1) * P, :].rearrange("a p c -> p a c"),
                    ut[:],
                )
                bt = wp.tile([P, w], bf16, tag="bt")
                nc.vector.tensor_scalar(
                    out=bt[:, :], in0=upsum[:, 3, :],
                    scalar1=dis[:, k : k + 1], scalar2=None,
                    op0=mybir.AluOpType.mult,
                )
                nc.sync.dma_start(
                    blk_rows[k * P : (k + 1) * P, :].rearrange(
                        "(a p) c -> p (a c)", p=P
                    ),
                    bt[:],
                )

        def allgather(blk, tab):
            nc.gpsimd.collective_compute(
                "AllGather",
                mybir.AluOpType.bypass,
                replica_groups=[list(range(m))],
                ins=[blk.ap().opt()],
                outs=[tab.ap().opt()],
            )

        # ---- one hop ----
        def hop(ls, idx_s, seg_s, tab, u_hbm, blk_rows, final, l):
            w = ls.w
            segbase = 0
            qrr = [0]
            for g in range(ng):
                tcnt = min(GT, T - g * GT)
                gath = {}
                for buck, w0, glen in ls.calls[g]:
                    gt = gp.tile([P, GCH, 128], bf16, tag="gath")
                    nc.gpsimd.dma_gather(
                        out_ap=gt[:, :glen, :],
                        in_ap=tab[buck * ls.buckrows : (buck + 1) * ls.buckrows, :],
                        idxs_ap=idx_s[:, w0 * 8 : (w0 + glen) * 8],
                        num_idxs=glen * P,
                        num_idxs_reg=glen * P,
                        elem_size=128,
                        queue_num=qrr[0] % NQ,
                    )
                    qrr[0] += 1
                    for j in range(glen):
                        gath[w0 + j] = (gt, j)

                psum = psa.tile([P, GT, w], f32, space="PSUM", tag="apsum")
                started = {e[1] for e in ls.pairs[g] if e[4]}
                for ttl in range(tcnt):
                    if ttl not in started:
                        nc.vector.memset(psum[:, ttl, :], 0.0)
                oneh = None
                npair_g = len(ls.pairs[g])
                for i, (k, ttl, par, cc, st, sp) in enumerate(ls.pairs[g]):
                    opos = i % OB
                    if opos == 0:
                        olen = min(OB, npair_g - i)
                        oneh = op.tile([P, OB, P], bf16, tag="oneh")
                        nc.vector.tensor_tensor(
                            out=oneh[:, :olen, :],
                            in0=iota_s[:].rearrange("p (a q) -> p a q", q=P)[
                                :, :olen, :
                            ],
                            in1=seg_s[:, segbase + i : segbase + i + olen]
                            .to_broadcast([P, olen, P]),
                            op=mybir.AluOpType.is_equal,
                        )
                    gt, slot = gath[k]
                    nc.tensor.matmul(
                        out=psum[:, ttl, :],
                        lhsT=oneh[:, opos, :],
                        rhs=gt[:, slot, par * w : (par + 1) * w],
                        start=st, stop=sp,
                    )
                segbase += npair_g

                # ---- writeout ----
                sl = slice(g * GT, g * GT + tcnt)
                rows = slice(g * GT * P, (g * GT + tcnt) * P)
                uti = up.tile([P, GT, w], f32, tag="uti")
                nc.sync.dma_start(
                    uti[:, :tcnt, :],
                    u_hbm[0 if final else ls.j, rows, :].rearrange(
                        "(a p) c -> p a c", p=P
                    ),
                )
                wt = wp.tile([P, GT, w], f32, tag="wt")
                nc.vector.tensor_tensor(
                    out=wt[:, :tcnt, :],
                    in0=psum[:, :tcnt, :],
                    in1=(ndis if final else ndis2)[:, sl].to_broadcast(
                        [P, tcnt, w]
                    ),
                    op=mybir.AluOpType.mult,
                )
                nc.vector.tensor_add(
                    wt[:, :tcnt, :], wt[:, :tcnt, :], uti[:, :tcnt, :]
                )
                if not final:
                    bt = ep.tile([P, GT, w], bf16, tag="bt")
                    nc.vector.tensor_copy(bt[:, :tcnt, :], wt[:, :tcnt, :])
                    nc.sync.dma_start(
                        blk_rows[rows, :].rearrange("(a p) c -> p a c", p=P),
                        bt[:, :tcnt, :],
                    )
                elif l == 1:
                    nc.vector.tensor_scalar(
                        out=wt[:, :tcnt, :], in0=wt[:, :tcnt, :],
                        scalar1=0.0, scalar2=None, op0=mybir.AluOpType.max,
                    )
                    bt = ep.tile([P, GT, w], bf16, tag="btr")
                    nc.vector.tensor_copy(bt[:, :tcnt, :], wt[:, :tcnt, :])
                    nc.sync.dma_start(
                        h_t[rows, :].rearrange("(a p) c -> p a c", p=P),
                        bt[:, :tcnt, :],
                    )
                else:
                    nc.vector.tensor_copy(lsm[:, sl, :], wt[:, :tcnt, :])

        lsm = const.tile([P, T, OUT], f32)

        # ================= layer 1 =================
        prologue(x_t, CIN, HID, v1, u1, rows_view(blks[(1, 3)], HID), b1s,
                 transpose_src=False)
        for j in (2, 1, 0):
            allgather(blks[(1, j + 1)], tabs[(1, j + 1)])
            l1.j = j
            hop(l1, idx1_s, seg1_s, tabs[(1, j + 1)], u1,
                rows_view(blks[(1, j)], HID) if j else None,
                final=(j == 0), l=1)

        if dbg:
            nc.sync.dma_start(dbg_blk[:, :], blks[(1, 3)][:, :])
            nc.sync.dma_start(dbg_tab[:, :], tabs[(1, 3)][:, :])
            nc.sync.dma_start(dbg_u1[:, :, :], u1[:, :, :])
            nc.sync.dma_start(dbg_h[:, :], h_t[:, :])
            nc.sync.dma_start(dbg_blk2[:, :], blks[(1, 2)][:, :])

        # ================= layer 2 =================
        prologue(h_t, HID, OUT, v2, u2, rows_view(blks[(2, 3)], OUT), b2s,
                 transpose_src=True)
        for j in (2, 1, 0):
            allgather(blks[(2, j + 1)], tabs[(2, j + 1)])
            l2.j = j
            hop(l2, idx2_s, seg2_s, tabs[(2, j + 1)], u2,
                rows_view(blks[(2, j)], OUT) if j else None,
                final=(j == 0), l=2)

        # ---- batched log_softmax epilogue over lsm [P, T, OUT] ----
        red = const.tile([P, T], f32)
        nc.vector.tensor_reduce(
            out=red[:], in_=lsm[:, :, :], axis=mybir.AxisListType.X,
            op=mybir.AluOpType.max,
        )
        nc.vector.tensor_tensor(
            out=lsm[:, :, :], in0=lsm[:, :, :],
            in1=red[:].to_broadcast([P, T, OUT]),
            op=mybir.AluOpType.subtract,
        )
        ex = const.tile([P, T, OUT], bf16)
        nc.scalar.activation(ex[:], lsm[:, :, :], mybir.ActivationFunctionType.Exp)
        nc.vector.tensor_reduce(
            out=red[:], in_=ex[:, :, :], axis=mybir.AxisListType.X,
            op=mybir.AluOpType.add,
        )
        nc.scalar.activation(red[:], red[:], mybir.ActivationFunctionType.Ln)
        nc.vector.tensor_tensor(
            out=lsm[:, :, :], in0=lsm[:, :, :],
            in1=red[:].to_broadcast([P, T, OUT]),
            op=mybir.AluOpType.subtract,
        )
        nc.sync.dma_start(
            y_t[:, :].rearrange("(a p) c -> p a c", p=P), lsm[:, :, :]
        )

    nc.compile()
    return nc


def make_in_maps(cfg: Cfg, inputs: dict, idxseg, degt_all):
    idx1, seg1, idx2, seg2 = idxseg
    import ml_dtypes

    x = np.asarray(inputs["x"], dtype=np.float32)
    maps = []
    for c in range(cfg.m):
        xb = np.zeros((cfg.bp, CIN), dtype=np.float32)
        xb[: cfg.b] = x[c * cfg.b : (c + 1) * cfg.b]
        xT = np.ascontiguousarray(xb.T).astype(ml_dtypes.bfloat16)
        maps.append(
            {
                "xT_blk": xT,
                "W1": np.asarray(inputs["W1"], dtype=np.float32),
                "b1": np.asarray(inputs["b1"], dtype=np.float32),
                "W2": np.asarray(inputs["W2"], dtype=np.float32),
                "b2": np.asarray(inputs["b2"], dtype=np.float32),
                "deg_t": degt_all[c],
                "idx1": idx1[c],
                "seg1": seg1[c],
                "idx2": idx2[c],
                "seg2": seg2[c],
            }
        )
    return maps


def kernel(**inputs) -> np.ndarray:
    from concourse import bass_utils

    cfg = Cfg(n=N_NODES, m=N_CORES)
    cfg.finish()
    edge_index = np.asarray(inputs["edge_index"])
    idxseg, degt_all = preprocess(edge_index, cfg)
    nc = build_program(cfg)
    in_maps = make_in_maps(cfg, inputs, idxseg, degt_all)
    res = bass_utils.run_bass_kernel_spmd(nc, in_maps, core_ids=list(range(cfg.m)))
    out = np.concatenate(
        [res.results[c]["y_blk"][: cfg.b] for c in range(cfg.m)], axis=0
    )
    return out.astype(np.float32)
